# revision 52
# baseline (speedup 1.0000x reference)
"""RNN-T Joiner kernel for 8 Trainium2 NeuronCores.

out[b,t,u,:] = tanh(enc[b,t,:] + pred[b,u,:]) @ W.T + b

Sharding: (b, t-half) per core -> core i handles batch i//2, t-range
[200*(i%2), 200*(i%2)+200). Per core 20000 (t,u) cells x 512 out.

Per-core pipeline (all operands bf16, accumulate fp32 in PSUM):
  - DVE: broadcast-add encT[:,t] + predT[:,u] -> logitT [c, (t,u)] bf16
  - ACT: tanh in place (bf16)
  - PE:  psum[cells, v] += logitT[c, cells].T @ WT[c, v]  (bf16, N=512)
  - DVE/ACT (split): copy psum pair [128, 2x512] -> sbuf bf16
  - DMA: 2x128KB contiguous stores per pair
  - host: upcast bf16 -> f32 and add the per-v bias during the gather
    (constant 512-float epilogue folded into the unshard step)

PE work is the roofline: 628 matmuls x ~520cyc @ 2.4GHz ~= 136us. A few
uninitialized-operand warmup matmuls run during the input DMA so HAM
un-throttles (1.2 -> 2.4 GHz) before the real stream starts, and block
sizes ramp 8t -> 64t so the first real matmul isn't gated on a big
logit unit.
"""

import sys

sys.path.insert(0, "/opt/trn_rl_repo")

import numpy as np
import ml_dtypes

import concourse.bass as bass
import concourse.bacc as bacc
import concourse.mybir as mybir
from concourse.tile import TileContext
from concourse.bass_utils import run_bass_kernel_spmd

B, T, U, C, V = 4, 400, 100, 512, 512
NCORES = 8
TS = 200  # t per core (b,t-half sharding)
P = 128
CK = C // P  # 4 chunks of the contraction dim
CELLS = TS * U  # 20000 cells per core
F32 = mybir.dt.float32
BF16 = mybir.dt.bfloat16
BT16 = ml_dtypes.bfloat16

# t-extent of each logit block: ramp-in so block k's PE work always
# covers block k+1's logit production (else the PE idles >3.4us at a
# boundary and HAM re-throttles it to 1.2 GHz). <=32t keeps every
# elementwise op under ~3.5us so psum-evac copies never sit long behind
# a tanh/add in the engines' runtime FIFOs. Ragged tails (800=6x128+32
# etc.) waste ~1.1% of PE -- cheaper than one HAM dip.
BLOCKS = [8, 16, 24, 28, 32, 32, 32, 28]
assert sum(BLOCKS) == TS

N_FILLER_MM = 6  # HAM keep-alive matmuls after each of the two first blocks

# packed consts layout (columns of the [128, NCOL] bf16 tensor)
W_OFF = 0  # [ck, v] -> 4*512
ENC_OFF = W_OFF + CK * V  # [ck, t] -> 4*200
PRED_OFF = ENC_OFF + CK * TS  # [ck, u] -> 4*100
NCOL = PRED_OFF + CK * U  # 3248

N_WARMUP_MM = 9  # ~3.8us of cold-rate matmuls: enough to trip HAM
# (more would delay the real matmuls queued behind them in PE's FIFO)

# which evac pair-copies go to DVE (rest on ACT); DVE also does the
# broadcast-adds, ACT the tanhs. Measured: DVE pair-copy 1.2us, ACT
# 1.15us; 4 of 10 puts both engines near ~128us, under the PE's ~137us.
# (Quad-granularity evac saves ~11us of engine time but halves the PE's
# psum runahead and measured 17us WORSE -- keep pairs.)
DVE_EVAC_SLOTS = {0, 3, 5, 8}  # of 10
GRP = 2  # psum tiles (banks) per evac unit

_cache = {}


def _build():
    nc = bacc.Bacc("TRN2", target_bir_lowering=False, debug=False)
    consts = nc.declare_dram_parameter("consts", [P, NCOL], BF16, isOutput=False)
    out = nc.declare_dram_parameter("out", [CELLS, V], BF16, isOutput=True)

    with TileContext(nc) as tc:
        with (
            tc.tile_pool(name="warm", bufs=1) as warm_pool,
            tc.tile_pool(name="consts", bufs=1) as cpool,
            tc.tile_pool(name="logit", bufs=3) as logit_pool,
            tc.tile_pool(name="osb", bufs=8) as out_pool,
            tc.tile_pool(name="psum", bufs=4, space="PSUM") as psum_pool,
        ):
            # PE warmup: matmuls on a memset SBUF tile so they only wait
            # on the (tiny) memset; issue first so HAM goes 8/8 while
            # the consts DMA + first logit unit are still in flight.
            dummy = warm_pool.tile([P, P + V], BF16, tag="dummy")
            nc.vector.memset(dummy[:], 1.0)

            # warmup+filler matmuls are dep-free (memset dummy operands)
            # so they run the moment the PE is otherwise idle.
            def emit_warmup(n):
                wps = psum_pool.tile([P, GRP, V], F32, tag="ps")
                for _ in range(n):
                    nc.tensor.matmul(
                        wps[:, 0, :],
                        lhsT=dummy[:, :P],
                        rhs=dummy[:, P : P + V],
                        start=True,
                        stop=True,
                    )

            emit_warmup(N_WARMUP_MM)

            # enc/pred first so logit production starts before W lands
            cs = cpool.tile([P, NCOL], BF16, tag="cs")
            nc.sync.dma_start(
                out=cs[:, ENC_OFF:NCOL], in_=consts.ap()[:, ENC_OFF:NCOL]
            )
            nc.sync.dma_start(
                out=cs[:, W_OFF:ENC_OFF], in_=consts.ap()[:, W_OFF:ENC_OFF]
            )

            wview = cs[:, W_OFF:ENC_OFF].rearrange("p (ck v) -> p ck v", ck=CK)
            eview = cs[:, ENC_OFF:PRED_OFF].rearrange("p (ck t) -> p ck t", ck=CK)
            pview = cs[:, PRED_OFF:NCOL].rearrange("p (ck u) -> p ck u", ck=CK)

            ob = out.ap()  # [20000, 512]

            nblk = len(BLOCKS)
            tstarts = [sum(BLOCKS[:k]) for k in range(nblk)]
            lg_tiles = [[None] * CK for _ in range(nblk)]

            def emit_produce(k, ck, nsplit=1):
                tlen = BLOCKS[k]
                t0 = tstarts[k]
                lgt = logit_pool.tile([P, tlen * U], BF16, tag=f"lg{ck}")
                # nsplit>1: emit in t-halves so the first matmuls are
                # gated on a smaller tanh (start-of-kernel latency).
                step = tlen // nsplit
                for h in range(nsplit):
                    ha, hl = h * step, step if h < nsplit - 1 else tlen - (
                        nsplit - 1
                    ) * step
                    lg3 = lgt[:, ha * U : (ha + hl) * U].rearrange(
                        "p (t u) -> p t u", t=hl
                    )
                    e_col = (
                        eview[:, ck, t0 + ha : t0 + ha + hl]
                        .unsqueeze(2)
                        .broadcast_to([P, hl, U])
                    )
                    p_row = (
                        pview[:, ck, :].unsqueeze(1).broadcast_to([P, hl, U])
                    )
                    nc.vector.tensor_add(out=lg3, in0=e_col, in1=p_row)
                    nc.scalar.activation(
                        out=lgt[:, ha * U : (ha + hl) * U],
                        in_=lgt[:, ha * U : (ha + hl) * U],
                        func=mybir.ActivationFunctionType.Tanh,
                    )
                lg_tiles[k][ck] = lgt

            for ck in range(CK):
                emit_produce(0, ck, nsplit=2)

            unit_cnt = 0
            for k in range(nblk):
                cells = BLOCKS[k] * U
                c0 = tstarts[k] * U
                lg = lg_tiles[k]
                starts = list(range(0, cells, GRP * P))
                nunits = len(starts)
                # interleave next block's logit production into this
                # block's unit stream (program order IS the runtime FIFO
                # order per engine; this bounds head-of-line blocking of
                # evac copies behind adds/tanhs to one ~3.5us op).
                # ramp blocks (k<=2): production is the critical path, so
                # emit it earlier and in t-halves (finer DVE/ACT ops ->
                # the next block's first tiles unblock sooner).
                ramp = k <= 2
                prod_after = {}
                if k + 1 < nblk:
                    for ck in range(CK):
                        prod_after.setdefault(
                            max(0, (ck * nunits) // CK - (2 if ramp else 1)),
                            [],
                        ).append(ck)
                for ui, s in enumerate(starts):
                    ms = [
                        min(P, max(0, cells - s - j * P)) for j in range(GRP)
                    ]
                    ntile = sum(1 for m in ms if m > 0)
                    ps = psum_pool.tile([P, GRP, V], F32, tag="ps")
                    for j in range(ntile):
                        for ck in range(CK):
                            nc.tensor.matmul(
                                ps[: ms[j], j, :],
                                lhsT=lg[ck][:, s + j * P : s + j * P + ms[j]],
                                rhs=wview[:, ck, :],
                                start=(ck == 0),
                                stop=(ck == CK - 1),
                            )
                    osb = out_pool.tile([P, GRP, V], BF16, tag="osb")
                    # copy the whole spanned region in one op (a ragged
                    # last tile's garbage rows are simply never DMA'd)
                    if ntile == 1:
                        src = ps[: ms[0], 0, :]
                        dst = osb[: ms[0], 0, :]
                    else:
                        src = ps[:, :ntile, :]
                        dst = osb[:, :ntile, :]
                    if unit_cnt % 10 in DVE_EVAC_SLOTS:
                        nc.vector.tensor_copy(out=dst, in_=src)
                    else:
                        nc.scalar.copy(out=dst, in_=src)
                    unit_cnt += 1
                    # final units: use the (idle) ACT HWDGE ring so the
                    # last few stores drain in parallel with Sync's ring
                    dma_eng = (
                        nc.scalar
                        if (k == nblk - 1 and ui >= nunits - 2)
                        else nc.sync
                    )
                    for j in range(ntile):
                        dma_eng.dma_start(
                            out=ob[c0 + s + j * P : c0 + s + j * P + ms[j], :],
                            in_=osb[: ms[j], j, :],
                        )
                    for ck in prod_after.get(ui, ()):
                        emit_produce(k + 1, ck, nsplit=2 if ramp else 1)
                if k < 2:
                    emit_warmup(N_FILLER_MM)
    nc.compile()
    return nc


def _install_ntff_hook():
    """This image's antenv lacks axon_hooks, so bass_utils' trace=True path
    can't find the NTFF profile hook. Inject the module and wire the ctypes
    hook from trn_boot against the axon PJRT .so."""
    if "antenv.axon_hooks" in sys.modules:
        return
    import types

    holder = [None]
    mod = types.ModuleType("antenv.axon_hooks")
    mod.set_axon_ntff_profile_hook = lambda h: holder.__setitem__(0, h)
    mod.get_axon_ntff_profile_hook = lambda: holder[0]
    sys.modules["antenv.axon_hooks"] = mod
    try:
        sys.path.insert(0, "/root/.axon_site/trn_agent_boot")
        from trn_boot import _ntff_profile_via_ctypes

        mod.set_axon_ntff_profile_hook(
            _ntff_profile_via_ctypes("/opt/axon/libaxon_pjrt.so")
        )
    except Exception as e:  # degrade to no tracing
        print(f"NTFF hook install failed: {e}", file=sys.stderr)


def _run(in_maps, trace=False, tmpdir=None):
    if "nc" not in _cache:
        _cache["nc"] = _build()
    if trace:
        _install_ntff_hook()
    return run_bass_kernel_spmd(
        _cache["nc"], in_maps, list(range(NCORES)), trace=trace, tmpdir=tmpdir
    )


def make_in_maps(encoder_out, predictor_out, W, b):
    encoder_out = np.asarray(encoder_out, dtype=np.float32)
    predictor_out = np.asarray(predictor_out, dtype=np.float32)
    W = np.asarray(W, dtype=np.float32)

    base = np.empty((P, NCOL), BT16)
    # [p, ck, v] <- W[v, ck*P+p]
    base[:, W_OFF:ENC_OFF] = (
        W.reshape(V, CK, P).transpose(2, 1, 0).reshape(P, CK * V).astype(BT16)
    )

    in_maps = []
    for i in range(NCORES):
        bi, th = divmod(i, 2)
        m = base.copy()
        enc_s = encoder_out[bi, th * TS : (th + 1) * TS, :]  # [t, c]
        m[:, ENC_OFF:PRED_OFF] = (
            enc_s.reshape(TS, CK, P).transpose(2, 1, 0).reshape(P, -1).astype(BT16)
        )
        m[:, PRED_OFF:NCOL] = (
            predictor_out[bi]
            .reshape(U, CK, P)
            .transpose(2, 1, 0)
            .reshape(P, -1)
            .astype(BT16)
        )
        in_maps.append({"consts": m})
    return in_maps


def gather(res, b):
    """Unshard: per-core [20000, 512] bf16 -> [B, T, U, V] f32 (+ bias)."""
    b = np.asarray(b, dtype=np.float32)
    out = np.empty((B, T, U, V), np.float32)
    for i in range(NCORES):
        bi, th = divmod(i, 2)
        part = np.asarray(res.results[i]["out"]).reshape(TS, U, V)
        out[bi, th * TS : (th + 1) * TS] = part.astype(np.float32) + b
    return out


def kernel(encoder_out, predictor_out, W, b):
    in_maps = make_in_maps(encoder_out, predictor_out, W, b)
    res = _run(in_maps, trace=False)
    return gather(res, b)


# revision 54
# speedup vs baseline: 1.0071x; 1.0071x over previous
"""RNN-T Joiner kernel for 8 Trainium2 NeuronCores.

out[b,t,u,:] = tanh(enc[b,t,:] + pred[b,u,:]) @ W.T + b

Sharding: (b, t-half) per core -> core i handles batch i//2, t-range
[200*(i%2), 200*(i%2)+200). Per core 20000 (t,u) cells x 512 out.

Per-core pipeline (all operands bf16, accumulate fp32 in PSUM):
  - DVE: broadcast-add encT[:,t] + predT[:,u] -> logitT [c, (t,u)] bf16
  - ACT: tanh in place (bf16)
  - PE:  psum[cells, v] += logitT[c, cells].T @ WT[c, v]  (bf16, N=512)
  - DVE/ACT (split): copy psum pair [128, 2x512] -> sbuf bf16
  - DMA: 2x128KB contiguous stores per pair
  - host: upcast bf16 -> f32 and add the per-v bias during the gather
    (constant 512-float epilogue folded into the unshard step)

PE work is the roofline: 628 matmuls x ~520cyc @ 2.4GHz ~= 136us. A few
uninitialized-operand warmup matmuls run during the input DMA so HAM
un-throttles (1.2 -> 2.4 GHz) before the real stream starts, and block
sizes ramp 8t -> 64t so the first real matmul isn't gated on a big
logit unit.
"""

import sys

sys.path.insert(0, "/opt/trn_rl_repo")

import numpy as np
import ml_dtypes

import concourse.bass as bass
import concourse.bacc as bacc
import concourse.mybir as mybir
from concourse.tile import TileContext
from concourse.bass_utils import run_bass_kernel_spmd

B, T, U, C, V = 4, 400, 100, 512, 512
NCORES = 8
TS = 200  # t per core (b,t-half sharding)
P = 128
CK = C // P  # 4 chunks of the contraction dim
CELLS = TS * U  # 20000 cells per core
F32 = mybir.dt.float32
BF16 = mybir.dt.bfloat16
BT16 = ml_dtypes.bfloat16

# t-extent of each logit block: ramp-in so block k's PE work always
# covers block k+1's logit production (else the PE idles >3.4us at a
# boundary and HAM re-throttles it to 1.2 GHz). <=32t keeps every
# elementwise op under ~3.5us so psum-evac copies never sit long behind
# a tanh/add in the engines' runtime FIFOs. Ragged tails (800=6x128+32
# etc.) waste ~1.1% of PE -- cheaper than one HAM dip.
BLOCKS = [8, 16, 24, 28, 32, 32, 32, 28]
assert sum(BLOCKS) == TS

N_FILLER_MM = 6  # HAM keep-alive matmuls after each of the two first blocks

# packed consts layout (columns of the [128, NCOL] bf16 tensor)
W_OFF = 0  # [ck, v] -> 4*512
ENC_OFF = W_OFF + CK * V  # [ck, t] -> 4*200
PRED_OFF = ENC_OFF + CK * TS  # [ck, u] -> 4*100
NCOL = PRED_OFF + CK * U  # 3248

N_WARMUP_MM = 9  # ~3.8us of cold-rate matmuls: enough to trip HAM
# (more would delay the real matmuls queued behind them in PE's FIFO)

# which evac pair-copies go to DVE (rest on ACT); DVE also does the
# broadcast-adds, ACT the tanhs. Measured: DVE pair-copy 1.2us, ACT
# 1.15us; 4 of 10 puts both engines near ~128us, under the PE's ~137us.
# (Quad-granularity evac saves ~11us of engine time but halves the PE's
# psum runahead and measured 17us WORSE -- keep pairs.)
DVE_EVAC_SLOTS = {0, 3, 5, 8}  # of 10
GRP = 2  # psum tiles (banks) per evac unit

_cache = {}


def _build():
    nc = bacc.Bacc("TRN2", target_bir_lowering=False, debug=False)
    consts = nc.declare_dram_parameter("consts", [P, NCOL], BF16, isOutput=False)
    out = nc.declare_dram_parameter("out", [CELLS, V], BF16, isOutput=True)

    with TileContext(nc) as tc:
        with (
            tc.tile_pool(name="warm", bufs=1) as warm_pool,
            tc.tile_pool(name="consts", bufs=1) as cpool,
            tc.tile_pool(name="logit", bufs=3) as logit_pool,
            tc.tile_pool(name="osb", bufs=8) as out_pool,
            tc.tile_pool(name="psum", bufs=4, space="PSUM") as psum_pool,
        ):
            # PE warmup: matmuls on a memset SBUF tile so they only wait
            # on the (tiny) memset; issue first so HAM goes 8/8 while
            # the consts DMA + first logit unit are still in flight.
            dummy = warm_pool.tile([P, P + V], BF16, tag="dummy")
            nc.vector.memset(dummy[:], 1.0)

            # warmup+filler matmuls are dep-free (memset dummy operands)
            # so they run the moment the PE is otherwise idle.
            def emit_warmup(n):
                wps = psum_pool.tile([P, GRP, V], F32, tag="ps")
                for _ in range(n):
                    nc.tensor.matmul(
                        wps[:, 0, :],
                        lhsT=dummy[:, :P],
                        rhs=dummy[:, P : P + V],
                        start=True,
                        stop=True,
                    )

            emit_warmup(N_WARMUP_MM)

            # enc/pred first so logit production starts before W lands
            cs = cpool.tile([P, NCOL], BF16, tag="cs")
            nc.sync.dma_start(
                out=cs[:, ENC_OFF:NCOL], in_=consts.ap()[:, ENC_OFF:NCOL]
            )
            nc.sync.dma_start(
                out=cs[:, W_OFF:ENC_OFF], in_=consts.ap()[:, W_OFF:ENC_OFF]
            )

            wview = cs[:, W_OFF:ENC_OFF].rearrange("p (ck v) -> p ck v", ck=CK)
            eview = cs[:, ENC_OFF:PRED_OFF].rearrange("p (ck t) -> p ck t", ck=CK)
            pview = cs[:, PRED_OFF:NCOL].rearrange("p (ck u) -> p ck u", ck=CK)

            ob = out.ap()  # [20000, 512]

            nblk = len(BLOCKS)
            tstarts = [sum(BLOCKS[:k]) for k in range(nblk)]
            lg_tiles = [[None] * CK for _ in range(nblk)]

            def emit_produce(k, ck, nsplit=1):
                tlen = BLOCKS[k]
                t0 = tstarts[k]
                lgt = logit_pool.tile([P, tlen * U], BF16, tag=f"lg{ck}")
                # nsplit>1: emit in t-halves so the first matmuls are
                # gated on a smaller tanh (start-of-kernel latency).
                step = tlen // nsplit
                for h in range(nsplit):
                    ha, hl = h * step, step if h < nsplit - 1 else tlen - (
                        nsplit - 1
                    ) * step
                    lg3 = lgt[:, ha * U : (ha + hl) * U].rearrange(
                        "p (t u) -> p t u", t=hl
                    )
                    e_col = (
                        eview[:, ck, t0 + ha : t0 + ha + hl]
                        .unsqueeze(2)
                        .broadcast_to([P, hl, U])
                    )
                    p_row = (
                        pview[:, ck, :].unsqueeze(1).broadcast_to([P, hl, U])
                    )
                    nc.vector.tensor_add(out=lg3, in0=e_col, in1=p_row)
                    nc.scalar.activation(
                        out=lgt[:, ha * U : (ha + hl) * U],
                        in_=lgt[:, ha * U : (ha + hl) * U],
                        func=mybir.ActivationFunctionType.Tanh,
                    )
                lg_tiles[k][ck] = lgt

            for ck in range(CK):
                emit_produce(0, ck, nsplit=2)

            unit_cnt = 0
            for k in range(nblk):
                cells = BLOCKS[k] * U
                c0 = tstarts[k] * U
                lg = lg_tiles[k]
                starts = list(range(0, cells, GRP * P))
                nunits = len(starts)
                # interleave next block's logit production into this
                # block's unit stream (program order IS the runtime FIFO
                # order per engine; this bounds head-of-line blocking of
                # evac copies behind adds/tanhs to one ~3.5us op).
                prod_after = {}
                if k + 1 < nblk:
                    for ck in range(CK):
                        prod_after.setdefault(
                            max(0, (ck * nunits) // CK - 1), []
                        ).append(ck)
                for ui, s in enumerate(starts):
                    ms = [
                        min(P, max(0, cells - s - j * P)) for j in range(GRP)
                    ]
                    ntile = sum(1 for m in ms if m > 0)
                    ps = psum_pool.tile([P, GRP, V], F32, tag="ps")
                    for j in range(ntile):
                        for ck in range(CK):
                            nc.tensor.matmul(
                                ps[: ms[j], j, :],
                                lhsT=lg[ck][:, s + j * P : s + j * P + ms[j]],
                                rhs=wview[:, ck, :],
                                start=(ck == 0),
                                stop=(ck == CK - 1),
                            )
                    osb = out_pool.tile([P, GRP, V], BF16, tag="osb")
                    # copy the whole spanned region in one op (a ragged
                    # last tile's garbage rows are simply never DMA'd)
                    if ntile == 1:
                        src = ps[: ms[0], 0, :]
                        dst = osb[: ms[0], 0, :]
                    else:
                        src = ps[:, :ntile, :]
                        dst = osb[:, :ntile, :]
                    if unit_cnt % 10 in DVE_EVAC_SLOTS:
                        nc.vector.tensor_copy(out=dst, in_=src)
                    else:
                        nc.scalar.copy(out=dst, in_=src)
                    unit_cnt += 1
                    # final units: use the (idle) ACT HWDGE ring so the
                    # last few stores drain in parallel with Sync's ring
                    dma_eng = (
                        nc.scalar
                        if (k == nblk - 1 and ui >= nunits - 2)
                        else nc.sync
                    )
                    for j in range(ntile):
                        dma_eng.dma_start(
                            out=ob[c0 + s + j * P : c0 + s + j * P + ms[j], :],
                            in_=osb[: ms[j], j, :],
                        )
                    for ck in prod_after.get(ui, ()):
                        emit_produce(k + 1, ck)
                if k < 2:
                    emit_warmup(N_FILLER_MM)
    nc.compile()
    return nc


def _install_ntff_hook():
    """This image's antenv lacks axon_hooks, so bass_utils' trace=True path
    can't find the NTFF profile hook. Inject the module and wire the ctypes
    hook from trn_boot against the axon PJRT .so."""
    if "antenv.axon_hooks" in sys.modules:
        return
    import types

    holder = [None]
    mod = types.ModuleType("antenv.axon_hooks")
    mod.set_axon_ntff_profile_hook = lambda h: holder.__setitem__(0, h)
    mod.get_axon_ntff_profile_hook = lambda: holder[0]
    sys.modules["antenv.axon_hooks"] = mod
    try:
        sys.path.insert(0, "/root/.axon_site/trn_agent_boot")
        from trn_boot import _ntff_profile_via_ctypes

        mod.set_axon_ntff_profile_hook(
            _ntff_profile_via_ctypes("/opt/axon/libaxon_pjrt.so")
        )
    except Exception as e:  # degrade to no tracing
        print(f"NTFF hook install failed: {e}", file=sys.stderr)


def _run(in_maps, trace=False, tmpdir=None):
    if "nc" not in _cache:
        _cache["nc"] = _build()
    if trace:
        _install_ntff_hook()
    return run_bass_kernel_spmd(
        _cache["nc"], in_maps, list(range(NCORES)), trace=trace, tmpdir=tmpdir
    )


def make_in_maps(encoder_out, predictor_out, W, b):
    encoder_out = np.asarray(encoder_out, dtype=np.float32)
    predictor_out = np.asarray(predictor_out, dtype=np.float32)
    W = np.asarray(W, dtype=np.float32)

    base = np.empty((P, NCOL), BT16)
    # [p, ck, v] <- W[v, ck*P+p]
    base[:, W_OFF:ENC_OFF] = (
        W.reshape(V, CK, P).transpose(2, 1, 0).reshape(P, CK * V).astype(BT16)
    )

    in_maps = []
    for i in range(NCORES):
        bi, th = divmod(i, 2)
        m = base.copy()
        enc_s = encoder_out[bi, th * TS : (th + 1) * TS, :]  # [t, c]
        m[:, ENC_OFF:PRED_OFF] = (
            enc_s.reshape(TS, CK, P).transpose(2, 1, 0).reshape(P, -1).astype(BT16)
        )
        m[:, PRED_OFF:NCOL] = (
            predictor_out[bi]
            .reshape(U, CK, P)
            .transpose(2, 1, 0)
            .reshape(P, -1)
            .astype(BT16)
        )
        in_maps.append({"consts": m})
    return in_maps


def gather(res, b):
    """Unshard: per-core [20000, 512] bf16 -> [B, T, U, V] f32 (+ bias)."""
    b = np.asarray(b, dtype=np.float32)
    out = np.empty((B, T, U, V), np.float32)
    for i in range(NCORES):
        bi, th = divmod(i, 2)
        part = np.asarray(res.results[i]["out"]).reshape(TS, U, V)
        out[bi, th * TS : (th + 1) * TS] = part.astype(np.float32) + b
    return out


def kernel(encoder_out, predictor_out, W, b):
    in_maps = make_in_maps(encoder_out, predictor_out, W, b)
    res = _run(in_maps, trace=False)
    return gather(res, b)


# revision 55
# speedup vs baseline: 1.0172x; 1.0101x over previous
"""RNN-T Joiner kernel for 8 Trainium2 NeuronCores.

out[b,t,u,:] = tanh(enc[b,t,:] + pred[b,u,:]) @ W.T + b

Sharding: (b, t-half) per core -> core i handles batch i//2, t-range
[200*(i%2), 200*(i%2)+200). Per core 20000 (t,u) cells x 512 out.

Per-core pipeline (all operands bf16, accumulate fp32 in PSUM):
  - DVE: broadcast-add encT[:,t] + predT[:,u] -> logitT [c, (t,u)] bf16
  - ACT: tanh in place (bf16)
  - PE:  psum[cells, v] += logitT[c, cells].T @ WT[c, v]  (bf16, N=512)
  - DVE/ACT (split): copy psum pair [128, 2x512] -> sbuf bf16
  - DMA: 2x128KB contiguous stores per pair
  - host: upcast bf16 -> f32 and add the per-v bias during the gather
    (constant 512-float epilogue folded into the unshard step)

PE work is the roofline: 628 matmuls x ~520cyc @ 2.4GHz ~= 136us. A few
uninitialized-operand warmup matmuls run during the input DMA so HAM
un-throttles (1.2 -> 2.4 GHz) before the real stream starts, and block
sizes ramp 8t -> 64t so the first real matmul isn't gated on a big
logit unit.
"""

import sys

sys.path.insert(0, "/opt/trn_rl_repo")

import numpy as np
import ml_dtypes

import concourse.bass as bass
import concourse.bacc as bacc
import concourse.mybir as mybir
from concourse.tile import TileContext
from concourse.bass_utils import run_bass_kernel_spmd

B, T, U, C, V = 4, 400, 100, 512, 512
NCORES = 8
TS = 200  # t per core (b,t-half sharding)
P = 128
CK = C // P  # 4 chunks of the contraction dim
CELLS = TS * U  # 20000 cells per core
F32 = mybir.dt.float32
BF16 = mybir.dt.bfloat16
BT16 = ml_dtypes.bfloat16

# t-extent of each logit block: ramp-in so block k's PE work always
# covers block k+1's logit production (else the PE idles >3.4us at a
# boundary and HAM re-throttles it to 1.2 GHz). <=32t keeps every
# elementwise op under ~3.5us so psum-evac copies never sit long behind
# a tanh/add in the engines' runtime FIFOs. Ragged tails (800=6x128+32
# etc.) waste ~1.1% of PE -- cheaper than one HAM dip.
BLOCKS = [8, 16, 24, 28, 32, 32, 32, 28]
assert sum(BLOCKS) == TS

N_FILLER_MM = 6  # HAM keep-alive matmuls after each of the two first blocks

# packed consts layout (columns of the [128, NCOL] bf16 tensor)
W_OFF = 0  # [ck, v] -> 4*512
ENC_OFF = W_OFF + CK * V  # [ck, t] -> 4*200
PRED_OFF = ENC_OFF + CK * TS  # [ck, u] -> 4*100
NCOL = PRED_OFF + CK * U  # 3248

N_WARMUP_MM = 9  # ~3.8us of cold-rate matmuls: enough to trip HAM
# (more would delay the real matmuls queued behind them in PE's FIFO)

# which evac pair-copies go to DVE (rest on ACT); DVE also does the
# broadcast-adds, ACT the tanhs. Measured: DVE pair-copy 1.2us, ACT
# 1.15us; 4 of 10 puts both engines near ~128us, under the PE's ~137us.
# (Quad-granularity evac saves ~11us of engine time but halves the PE's
# psum runahead and measured 17us WORSE -- keep pairs.)
DVE_EVAC_SLOTS = {0, 3, 5, 8}  # of 10
GRP = 2  # psum tiles (banks) per evac unit

_cache = {}


def _build():
    nc = bacc.Bacc("TRN2", target_bir_lowering=False, debug=False)
    consts = nc.declare_dram_parameter("consts", [P, NCOL], BF16, isOutput=False)
    out = nc.declare_dram_parameter("out", [CELLS, V], BF16, isOutput=True)

    with TileContext(nc) as tc:
        with (
            tc.tile_pool(name="warm", bufs=1) as warm_pool,
            tc.tile_pool(name="consts", bufs=1) as cpool,
            tc.tile_pool(name="logit", bufs=3) as logit_pool,
            tc.tile_pool(name="osb", bufs=8) as out_pool,
            tc.tile_pool(name="psum", bufs=4, space="PSUM") as psum_pool,
        ):
            # PE warmup: matmuls on a memset SBUF tile so they only wait
            # on the (tiny) memset; issue first so HAM goes 8/8 while
            # the consts DMA + first logit unit are still in flight.
            dummy = warm_pool.tile([P, P + V], BF16, tag="dummy")
            nc.vector.memset(dummy[:], 1.0)

            # warmup+filler matmuls are dep-free (memset dummy operands)
            # so they run the moment the PE is otherwise idle.
            def emit_warmup(n):
                wps = psum_pool.tile([P, GRP, V], F32, tag="ps")
                for _ in range(n):
                    nc.tensor.matmul(
                        wps[:, 0, :],
                        lhsT=dummy[:, :P],
                        rhs=dummy[:, P : P + V],
                        start=True,
                        stop=True,
                    )

            emit_warmup(N_WARMUP_MM)

            # enc/pred first so logit production starts before W lands
            cs = cpool.tile([P, NCOL], BF16, tag="cs")
            nc.sync.dma_start(
                out=cs[:, ENC_OFF:NCOL], in_=consts.ap()[:, ENC_OFF:NCOL]
            )
            nc.sync.dma_start(
                out=cs[:, W_OFF:ENC_OFF], in_=consts.ap()[:, W_OFF:ENC_OFF]
            )

            wview = cs[:, W_OFF:ENC_OFF].rearrange("p (ck v) -> p ck v", ck=CK)
            eview = cs[:, ENC_OFF:PRED_OFF].rearrange("p (ck t) -> p ck t", ck=CK)
            pview = cs[:, PRED_OFF:NCOL].rearrange("p (ck u) -> p ck u", ck=CK)

            ob = out.ap()  # [20000, 512]

            nblk = len(BLOCKS)
            tstarts = [sum(BLOCKS[:k]) for k in range(nblk)]
            lg_tiles = [[None] * CK for _ in range(nblk)]

            def emit_produce(k, ck, nsplit=1):
                tlen = BLOCKS[k]
                t0 = tstarts[k]
                lgt = logit_pool.tile([P, tlen * U], BF16, tag=f"lg{ck}")
                # nsplit>1: emit in t-halves so the first matmuls are
                # gated on a smaller tanh (start-of-kernel latency).
                step = tlen // nsplit
                for h in range(nsplit):
                    ha, hl = h * step, step if h < nsplit - 1 else tlen - (
                        nsplit - 1
                    ) * step
                    lg3 = lgt[:, ha * U : (ha + hl) * U].rearrange(
                        "p (t u) -> p t u", t=hl
                    )
                    e_col = (
                        eview[:, ck, t0 + ha : t0 + ha + hl]
                        .unsqueeze(2)
                        .broadcast_to([P, hl, U])
                    )
                    p_row = (
                        pview[:, ck, :].unsqueeze(1).broadcast_to([P, hl, U])
                    )
                    nc.vector.tensor_add(out=lg3, in0=e_col, in1=p_row)
                    nc.scalar.activation(
                        out=lgt[:, ha * U : (ha + hl) * U],
                        in_=lgt[:, ha * U : (ha + hl) * U],
                        func=mybir.ActivationFunctionType.Tanh,
                    )
                lg_tiles[k][ck] = lgt

            for ck in range(CK):
                emit_produce(0, ck, nsplit=2)

            unit_cnt = 0
            for k in range(nblk):
                cells = BLOCKS[k] * U
                c0 = tstarts[k] * U
                lg = lg_tiles[k]
                starts = list(range(0, cells, GRP * P))
                nunits = len(starts)
                # interleave next block's logit production into this
                # block's unit stream (program order IS the runtime FIFO
                # order per engine; this bounds head-of-line blocking of
                # evac copies behind adds/tanhs to one ~3.5us op).
                prod_after = {}
                if k + 1 < nblk:
                    for ck in range(CK):
                        prod_after.setdefault(
                            max(0, (ck * nunits) // CK - 1), []
                        ).append(ck)
                for ui, s in enumerate(starts):
                    ms = [
                        min(P, max(0, cells - s - j * P)) for j in range(GRP)
                    ]
                    ntile = sum(1 for m in ms if m > 0)
                    ps = psum_pool.tile([P, GRP, V], F32, tag="ps")
                    for j in range(ntile):
                        for ck in range(CK):
                            nc.tensor.matmul(
                                ps[: ms[j], j, :],
                                lhsT=lg[ck][:, s + j * P : s + j * P + ms[j]],
                                rhs=wview[:, ck, :],
                                start=(ck == 0),
                                stop=(ck == CK - 1),
                            )
                    osb = out_pool.tile([P, GRP, V], BF16, tag="osb")
                    # copy the whole spanned region in one op (a ragged
                    # last tile's garbage rows are simply never DMA'd)
                    if ntile == 1:
                        src = ps[: ms[0], 0, :]
                        dst = osb[: ms[0], 0, :]
                    else:
                        src = ps[:, :ntile, :]
                        dst = osb[:, :ntile, :]
                    if unit_cnt % 10 in DVE_EVAC_SLOTS:
                        nc.vector.tensor_copy(out=dst, in_=src)
                    else:
                        nc.scalar.copy(out=dst, in_=src)
                    unit_cnt += 1
                    for j in range(ntile):
                        nc.sync.dma_start(
                            out=ob[c0 + s + j * P : c0 + s + j * P + ms[j], :],
                            in_=osb[: ms[j], j, :],
                        )
                    for ck in prod_after.get(ui, ()):
                        emit_produce(k + 1, ck)
                if k < 2:
                    emit_warmup(N_FILLER_MM)
    nc.compile()
    return nc


def _install_ntff_hook():
    """This image's antenv lacks axon_hooks, so bass_utils' trace=True path
    can't find the NTFF profile hook. Inject the module and wire the ctypes
    hook from trn_boot against the axon PJRT .so."""
    if "antenv.axon_hooks" in sys.modules:
        return
    import types

    holder = [None]
    mod = types.ModuleType("antenv.axon_hooks")
    mod.set_axon_ntff_profile_hook = lambda h: holder.__setitem__(0, h)
    mod.get_axon_ntff_profile_hook = lambda: holder[0]
    sys.modules["antenv.axon_hooks"] = mod
    try:
        sys.path.insert(0, "/root/.axon_site/trn_agent_boot")
        from trn_boot import _ntff_profile_via_ctypes

        mod.set_axon_ntff_profile_hook(
            _ntff_profile_via_ctypes("/opt/axon/libaxon_pjrt.so")
        )
    except Exception as e:  # degrade to no tracing
        print(f"NTFF hook install failed: {e}", file=sys.stderr)


def _run(in_maps, trace=False, tmpdir=None):
    if "nc" not in _cache:
        _cache["nc"] = _build()
    if trace:
        _install_ntff_hook()
    return run_bass_kernel_spmd(
        _cache["nc"], in_maps, list(range(NCORES)), trace=trace, tmpdir=tmpdir
    )


def make_in_maps(encoder_out, predictor_out, W, b):
    encoder_out = np.asarray(encoder_out, dtype=np.float32)
    predictor_out = np.asarray(predictor_out, dtype=np.float32)
    W = np.asarray(W, dtype=np.float32)

    base = np.empty((P, NCOL), BT16)
    # [p, ck, v] <- W[v, ck*P+p]
    base[:, W_OFF:ENC_OFF] = (
        W.reshape(V, CK, P).transpose(2, 1, 0).reshape(P, CK * V).astype(BT16)
    )

    in_maps = []
    for i in range(NCORES):
        bi, th = divmod(i, 2)
        m = base.copy()
        enc_s = encoder_out[bi, th * TS : (th + 1) * TS, :]  # [t, c]
        m[:, ENC_OFF:PRED_OFF] = (
            enc_s.reshape(TS, CK, P).transpose(2, 1, 0).reshape(P, -1).astype(BT16)
        )
        m[:, PRED_OFF:NCOL] = (
            predictor_out[bi]
            .reshape(U, CK, P)
            .transpose(2, 1, 0)
            .reshape(P, -1)
            .astype(BT16)
        )
        in_maps.append({"consts": m})
    return in_maps


def gather(res, b):
    """Unshard: per-core [20000, 512] bf16 -> [B, T, U, V] f32 (+ bias)."""
    b = np.asarray(b, dtype=np.float32)
    out = np.empty((B, T, U, V), np.float32)
    for i in range(NCORES):
        bi, th = divmod(i, 2)
        part = np.asarray(res.results[i]["out"]).reshape(TS, U, V)
        out[bi, th * TS : (th + 1) * TS] = part.astype(np.float32) + b
    return out


def kernel(encoder_out, predictor_out, W, b):
    in_maps = make_in_maps(encoder_out, predictor_out, W, b)
    res = _run(in_maps, trace=False)
    return gather(res, b)
